# revision 27
# baseline (speedup 1.0000x reference)
"""Causal MHA (B=2, L=2048, D=1024, 16 heads, RoPE) on 8 Trainium2 NeuronCores.

Strategy: tensor-parallel over heads (2 heads/core).
 - Host: transpose x -> x^T, build per-core W_qkv^T slices (q cols pre-scaled by
   1/sqrt(hd)), W_out^T, and RoPE cos/sin tables ([128, L], shared by both
   batches); everything fp16 for matmul operands (fp32 PSUM accumulation).
 - Device per core, fully software-pipelined at emission level:
     * inputs stream in consumption order (wq, cos/sin, x^T quarter-tiles,
       wout last) so the first attention pass can start ~15us in;
     * QK projection bursts are interleaved with attention passes; RoPE
       multiplies read the projection PSUM tile directly (no ACT copy);
     * attention pass: S^T = K^T.T @ Q^T -> exp (ACT) -> PV; the previous
       pass's PV matmuls are interleaved between the current pass's ST
       matmuls so the PE fills the stalls while exp paces the pipeline;
     * ones-column in V' yields softmax denominators; no max subtraction.
 - AllToAll pipelined in 4 per-1024-token groups: each group's collective
   fires mid-attention as soon as both head-halves of its chunk pair are
   normalized, so the redistribution and most of the output projection
   overlap later attention passes; only the last group's (small) collective
   and 128-token projection remain in the tail.
 - Host: scatter the per-core [4 x 128 tokens, 1024] chunks into the full
   output.
"""

import numpy as np

import concourse.bass as bass
import concourse.mybir as mybir
import concourse.tile as tile
from concourse import bacc
from concourse.bass_utils import run_bass_kernel_spmd

B, L, D, NH, HD = 2, 2048, 1024, 16, 64
ROPE_BASE = 10000.0
N_CORES = 8
HPC = NH // N_CORES          # heads per core = 2
M = B * L                    # 4096 tokens
MCH = 512                    # m-chunk (proj free dim)
KT = D // 128                # 8 contraction tiles
QB = 512                     # q chunk in attention
KB = 128                     # k block in attention
NKB = L // KB                # 16
NQC = L // QB                # 4
XCW = 1024                   # x^T column tile width
NXC = M // XCW               # 4 column tiles per k-tile
OH = MCH // 2                # out-proj half width (256)

fp16 = mybir.dt.float16
fp32 = mybir.dt.float32
fp8 = mybir.dt.float8e4

_NC = None


def _build_nc():
    nc = bacc.Bacc("TRN2", target_bir_lowering=False, debug=False,
                   num_devices=N_CORES)

    xT = nc.dram_tensor("xT", [D, M], fp16, kind="ExternalInput").ap()
    wqkvT = nc.dram_tensor("wqkvT", [D, 384], fp16, kind="ExternalInput").ap()
    woutT = nc.dram_tensor("woutT", [D, D], fp16, kind="ExternalInput").ap()
    cosT = nc.dram_tensor("cosT", [128, L], fp16, kind="ExternalInput").ap()
    sinT = nc.dram_tensor("sinT", [128, L], fp16, kind="ExternalInput").ap()
    out = nc.dram_tensor("out", [MCH, D], fp16, kind="ExternalOutput").ap()

    # 4 pipelined AllToAll groups, one per 1024-token chunk pair: group G
    # covers global tokens [G*1024, (G+1)*1024); destination core d receives
    # [all 1024 h-dims, tokens G*1024 + d*128 .. +128] and projects them.
    cc_in = [nc.dram_tensor(f"cc_in{g}", [D, 128], fp16) for g in range(4)]
    cc_out = [nc.dram_tensor(f"cc_out{g}", [D, 128], fp16) for g in range(4)]

    with tile.TileContext(nc) as tc:
        with tc.tile_pool(name="persist", bufs=1) as per, \
             tc.tile_pool(name="weights", bufs=1) as wp:
            # ---- constants first: they run on gpsimd/scalar immediately,
            # before those queues jam behind the input DMA flood ----
            # triangular 0/1 mask: keep where q-col >= k-row
            mask0 = per.tile([128, QB], fp16, tag="mask0")
            nc.gpsimd.memset(mask0[:], 1.0)
            nc.gpsimd.affine_select(
                out=mask0[:], in_=mask0[:], compare_op=mybir.AluOpType.is_ge,
                fill=0.0, base=0, channel_multiplier=-1, pattern=[[1, QB]],
            )
            ebias = per.tile([128, 1], fp32, tag="ebias")
            nc.gpsimd.memset(ebias[:], -3.0)
            # preload the exp table set during the input-DMA wait so the
            # first real exp skips the ~2.7us ACT_TABLE_LOAD
            dexp = per.tile([1, 1], fp16, tag="dexp")
            nc.scalar.activation(dexp[:], mask0[0:1, 0:1],
                                 mybir.ActivationFunctionType.Exp)
            # ---- input DMAs, in consumption order ----
            wq = [wp.tile([128, 384], fp16, tag=f"wq{k}", name=f"wq{k}")
                  for k in range(KT)]
            for k in range(KT):
                nc.sync.dma_start(wq[k][:], wqkvT[k * 128:(k + 1) * 128, :])
            # cos/sin split: the first half covers RoPE for token chunks 0-1,
            # so the second half can load after the first x column group
            cos_t = wp.tile([128, L], fp16, tag="cos")
            sin_t = wp.tile([128, L], fp16, tag="sin")
            nc.sync.dma_start(cos_t[:, 0:XCW], cosT[:, 0:XCW])
            nc.sync.dma_start(sin_t[:, 0:XCW], sinT[:, 0:XCW])
            # x^T as 8 k-tiles x 4 column tiles, streamed column-major so the
            # first 1024 tokens land first
            # alternate trigger queues: each DMA trigger costs ~650ns of
            # queue time, and 42 serial triggers (not HBM bandwidth) were
            # pacing the later column groups
            xt = [[None] * NXC for _ in range(KT)]
            for c in range(NXC):
                for k in range(KT):
                    t = wp.tile([128, XCW], fp16, tag=f"xt{k}_{c}",
                                name=f"xt{k}_{c}")
                    eng = nc.sync if k % 2 == 0 else nc.scalar
                    eng.dma_start(
                        t[:], xT[k * 128:(k + 1) * 128, c * XCW:(c + 1) * XCW])
                    xt[k][c] = t
                if c == 0:
                    nc.sync.dma_start(cos_t[:, XCW:L], cosT[:, XCW:L])
                    nc.sync.dma_start(sin_t[:, XCW:L], sinT[:, XCW:L])
            wo = [wp.tile([128, D], fp16, tag=f"wo{k}", name=f"wo{k}")
                  for k in range(KT)]
            for k in range(KT):
                nc.scalar.dma_start(wo[k][:], woutT[k * 128:(k + 1) * 128, :])

            def xcol(k, gc):
                """x^T slice for global 512-token chunk gc, k-tile k."""
                return xt[k][gc // 2][:, (gc % 2) * MCH:(gc % 2 + 1) * MCH]

            def xcol128(k, mt):
                """x^T slice for global 128-token tile mt, k-tile k."""
                return xt[k][mt // 8][:, (mt % 8) * 128:(mt % 8 + 1) * 128]

            # Q^T/K^T per batch: rows 0-63 head0, 64-127 head1;
            # cols 0:L = Q, L:2L = K
            qku = [per.tile([128, 2 * L], fp16, tag=f"qku{b}", name=f"qku{b}")
                   for b in range(B)]
            # V' per batch: per k-tile 130 cols = [v_h0(64) | 1 | v_h1(64) | 1]
            vt = [per.tile([128, (L // 128) * 130], fp16, tag=f"vt{b}",
                           name=f"vt{b}") for b in range(B)]
            # fp8 copy for DoubleRow PV over full-block pairs; per k-tile 144
            # cols = 2 head slots of 72 = [v(64) | 1 | pad(7)] so the pack-dim
            # byte-step (144) stays 16B-aligned
            vt8 = [per.tile([128, (L // 128) * 144], fp8, tag=f"vt8{b}",
                            name=f"vt8{b}") for b in range(B)]
            for b in range(B):
                nc.gpsimd.memset(vt[b][:], 1.0)
                nc.gpsimd.memset(vt8[b][:], 1.0)

            with tc.tile_pool(name="pp_ps", bufs=2, space="PSUM") as pps, \
                 tc.tile_pool(name="st_ps", bufs=2, space="PSUM") as stps, \
                 tc.tile_pool(name="h_ps", bufs=2, space="PSUM") as hps, \
                 tc.tile_pool(name="pt_sb", bufs=25) as ptp, \
                 tc.tile_pool(name="rope_sb", bufs=3) as rsb, \
                 tc.tile_pool(name="norm_sb", bufs=2) as nsb, \
                 tc.tile_pool(name="op_sb", bufs=3) as osb:

                # PE warm-up on the memset mask tile (no DMA dependency);
                # covers dispatch latency until the first x tiles land, so
                # the projection burst runs at full clock
                for _ in range(30):
                    wt = pps.tile([128, MCH], fp32, tag="pp", name="warm")
                    nc.tensor.matmul(wt[:], mask0[:, 0:128], mask0[:],
                                     start=True, stop=True)
                # then warm-ups paced by the x-tile DMAs themselves: activity
                # tracks the arrivals, so HAM never sees a >3.4us idle window
                # before the projection starts
                for k in range(KT):
                    for _ in range(2):
                        wt = pps.tile([128, MCH], fp32, tag="pp", name="warmx")
                        nc.tensor.matmul(wt[:], xt[k][0][:, 0:128],
                                         xt[k][0][:, 0:MCH],
                                         start=True, stop=True)

                def emit_qk_chunk(b_p, lh, mcb, early=False):
                    """QKV q/k projection + RoPE for one 512-token chunk.
                    Rope SBUF-SBUF moves always ride the gpsimd queue: it is
                    idle during startup (the sync queue is jammed behind the
                    8MB x-input flood) and never queues behind exps."""
                    dma_eng = nc.gpsimd
                    gc = b_p * NQC + mcb
                    qkp = pps.tile([128, MCH], fp32, tag="pp", name="qkp")
                    for k in range(KT):
                        nc.tensor.matmul(
                            qkp[:], wq[k][:, lh * 128:(lh + 1) * 128],
                            xcol(k, gc), start=(k == 0), stop=(k == KT - 1))
                    cs = slice(mcb * MCH, (mcb + 1) * MCH)
                    a_t = rsb.tile([128, MCH], fp16, tag="a")
                    c_t = rsb.tile([128, MCH], fp16, tag="c")
                    nc.vector.tensor_mul(a_t[:], qkp[:], cos_t[:, cs])
                    nc.vector.tensor_mul(c_t[:], qkp[:], sin_t[:, cs])
                    tmp = rsb.tile([128, MCH], fp16, tag="tmp")
                    for g in range(4):  # swap 32-row halves
                        src = (g ^ 1) * 32
                        dma_eng.dma_start(
                            tmp[g * 32:(g + 1) * 32, :],
                            c_t[src:src + 32, :])
                    bcol = mcb * MCH
                    # head0: q rows 0-63, k rows 64-127
                    # head1: k rows 0-63, q rows 64-127
                    qrows = slice(0, 64) if lh == 0 else slice(64, 128)
                    krows = slice(64, 128) if lh == 0 else slice(0, 64)
                    drows = slice(lh * 64, (lh + 1) * 64)
                    nc.vector.tensor_add(
                        qku[b_p][drows, bcol:bcol + MCH],
                        a_t[qrows, :], tmp[qrows, :])
                    nc.vector.tensor_add(
                        tmp[krows, :], a_t[krows, :], tmp[krows, :])
                    dma_eng.dma_start(
                        qku[b_p][drows, L + bcol:L + bcol + MCH],
                        tmp[krows, :])

                def emit_v_tile(mt):
                    """V projection for one 128-token tile."""
                    vp = pps.tile([128, 128], fp32, tag="pp", name="vp")
                    for k in range(KT):
                        nc.tensor.matmul(
                            vp[:], xcol128(k, mt), wq[k][:, 256:384],
                            start=(k == 0), stop=(k == KT - 1))
                    b_, kt_ = mt // (L // 128), mt % (L // 128)
                    dst = vt[b_][:, kt_ * 130:kt_ * 130 + 130]
                    dst = dst.rearrange("p (g c) -> p g c", g=2)[:, :, 0:64]
                    nc.vector.tensor_copy(
                        dst, vp[:].rearrange("p (g c) -> p g c", g=2))
                    dst8 = vt8[b_][:, kt_ * 144:kt_ * 144 + 144]
                    dst8 = dst8.rearrange("p (g c) -> p g c", g=2)[:, :, 0:64]
                    nc.vector.tensor_copy(
                        dst8, vp[:].rearrange("p (g c) -> p g c", g=2))

                def emit_qk_pair(b_p, lh, pair):
                    early = (b_p == 0 and pair == 0)
                    for mcb in (2 * pair, 2 * pair + 1):
                        emit_qk_chunk(b_p, lh, mcb, early=early)

                def emit_v_half(b_p, pair):
                    for mt in range(b_p * 16 + pair * 8,
                                    b_p * 16 + pair * 8 + 8):
                        emit_v_tile(mt)

                def st_block(b_, lh, pas, ki):
                    """ST matmuls + exp + mask for one k-block; returns stash
                    entry."""
                    rows = slice(lh * 64, (lh + 1) * 64)
                    qcs = (2 * pas, 2 * pas + 1)
                    qlo = max(qcs[0], ki // (QB // KB))
                    pofs = []
                    for qc in range(qlo, qcs[1] + 1):
                        diag = (qc == ki // (QB // KB))
                        off = (ki % (QB // KB)) * KB if diag else 0
                        w = QB - off
                        # fixed 512-stride slots: each matmul stays in one
                        # PSUM bank; diag slack unread downstream
                        pofs.append((qc, (qc - qlo) * QB, w,
                                     qc * QB + off, off))
                    p0 = (qcs[1] + 1 - qlo) * QB
                    st = stps.tile([128, p0], fp32, tag="st", name="st")
                    pt = ptp.tile([128, p0], fp16, tag="pt", name="pt")
                    for qc, ps, w, qs, off in pofs:
                        nc.tensor.matmul(
                            st[:, ps:ps + w],
                            qku[b_][rows, L + ki * KB:L + (ki + 1) * KB],
                            qku[b_][rows, qs:qs + w],
                            start=True, stop=True)
                    # exp(x-3): uniform rescale (cancels in the softmax
                    # ratio) keeping values inside fp8e4 range for the
                    # DoubleRow path
                    nc.scalar.activation(
                        pt[:], st[:], mybir.ActivationFunctionType.Exp,
                        bias=ebias[:])
                    if ki // (QB // KB) == qlo:
                        # mask differs from 1 only in the first KB cols
                        w0 = min(pofs[0][2], KB)
                        nc.vector.tensor_mul(
                            pt[:, 0:w0], pt[:, 0:w0], mask0[:, 0:w0])
                    return (ki, pofs, pt)

                def norm_one(b_, lh, qc, ha):
                    """Normalize one q-chunk (denominator in ha row 64) and
                    ship h^T to its AllToAll group buffer. Emitted at the
                    accumulation's stop matmul, so the ACT row-move queues
                    only a few exps deep."""
                    dsb = nsb.tile([1, QB], fp32, tag="dsb")
                    nc.scalar.copy(dsb[:], ha[64:65, :])
                    recip = nsb.tile([1, QB], fp32, tag="recip")
                    nc.vector.reciprocal_approx_fast(recip[:], dsb[:])
                    rb = nsb.tile([64, QB], fp32, tag="rb")
                    nc.gpsimd.partition_broadcast(rb[:], recip[:])
                    ht = nsb.tile([64, QB], fp16, tag="ht")
                    nc.vector.tensor_mul(ht[:], ha[0:64, :], rb[:])
                    # group G, chunk half e: dest core d = 4e+c takes token
                    # cols [c*128, (c+1)*128); our 64 rows land at global
                    # h-dim rows d*128 + lh*64 of cc_in[G]
                    G = b_ * 2 + qc // 2
                    e = qc % 2
                    dst = cc_in[G].ap().rearrange(
                        "(d p) t -> p d t", p=128)[
                            lh * 64:lh * 64 + 64, 4 * e:4 * e + 4, :]
                    nc.sync.dma_start(
                        dst, ht[:, :].rearrange("p (c t) -> p c t", t=128))

                def pv_mms(b_, lh, pas, hacc, entry):
                    """PV matmuls for one stash entry of pass (b_, lh, pas)."""
                    ki, pofs, pt = entry
                    if pofs == "DR":
                        # fp8 DoubleRow: one matmul contracts k-blocks ki and
                        # ki+1 (256 rows) at 0.5 cycles/col
                        vsl8 = vt8[b_].rearrange("p (k c) -> p k c", c=144)[
                            :, ki:ki + 2, lh * 72:lh * 72 + 65]
                        p2 = pt.rearrange("p (j c) -> p j c", j=2)
                        for qc in (2, 3):
                            nc.tensor.matmul(
                                hacc[qc][:, 0:QB], vsl8,
                                p2[:, :, (qc - 2) * QB:(qc - 1) * QB],
                                start=(ki == 0), stop=False,
                                perf_mode=mybir.MatmulPerfMode.DoubleRow)
                        return
                    vsl = vt[b_][:, ki * 130 + lh * 65:ki * 130 + lh * 65 + 65]
                    for qc, ps, w, qs, off in pofs:
                        nc.tensor.matmul(
                            hacc[qc][:, off:off + w], vsl, pt[:, ps:ps + w],
                            start=(ki == 0),
                            stop=(ki == (qc + 1) * (QB // KB) - 1))

                def fire_group(G):
                    """Trigger the AllToAll for token group G (inputs are the
                    norm DMAs already emitted); runs async on the CC engine
                    while attention continues."""
                    nc.gpsimd.collective_compute(
                        "AllToAll", mybir.AluOpType.bypass,
                        replica_groups=[list(range(N_CORES))],
                        ins=[cc_in[G].ap().opt()],
                        outs=[cc_out[G].ap().opt()],
                    )

                htf_of = {}

                def emit_htf(G, eng=None):
                    """One-shot SBUF load of the received h^T for group G
                    (single 256KB DMA; waits on the group's AllToAll
                    semaphore inside the chosen DMA queue)."""
                    htf = osb.tile([128, 1024], fp16, tag="htf", name="htf")
                    src = cc_out[G].ap().rearrange("(g p) t -> p g t", p=128)
                    (eng or nc.sync).dma_start(
                        htf[:].rearrange("p (g t) -> p g t", t=128), src)
                    htf_of[G] = htf

                def emit_proj_half(G, dh, out_eng=None):
                    """Out-projection of group G's 128 tokens, dims half dh.
                    h tile stationary, Wout moving -> stream-bound N=512."""
                    htf = htf_of[G]
                    op = pps.tile([128, 512], fp32, tag="pp", name="proj")
                    for k in range(KT):
                        nc.tensor.matmul(
                            op[:], htf[:, k * 128:(k + 1) * 128],
                            wo[k][:, dh * 512:(dh + 1) * 512],
                            start=(k == 0), stop=(k == KT - 1))
                    ot = nsb.tile([128, 512], fp16, tag="ot")
                    nc.vector.tensor_copy(ot[:], op[:])
                    (out_eng or nc.sync).dma_start(
                        out[G * 128:(G + 1) * 128, dh * 512:(dh + 1) * 512],
                        ot[:])

                # ---- interleaved projection pieces + attention passes ----
                # lh-adjacent pass order completes each 1024-token group as
                # early as possible so its AllToAll overlaps later passes; qk
                # pieces land just before the pass that needs them so the
                # first ST only queues behind 16 projection matmuls
                # b1's first qk pair is pulled one slot earlier so its RoPE
                # vector/gpsimd work overlaps the (vector-light) DR pass p4
                # instead of stalling p5's first STs
                events = [("qk", 0, 0, 0), ("pass", 0, 0, 0),
                          ("qk", 0, 1, 0), ("v", 0, 0),
                          ("pass", 0, 1, 0),
                          ("qk", 0, 0, 1), ("v", 0, 1),
                          ("pass", 0, 0, 1),
                          ("qk", 0, 1, 1), ("qk", 1, 0, 0),
                          ("pass", 0, 1, 1),
                          ("pass", 1, 0, 0),
                          ("qk", 1, 1, 0), ("v", 1, 0),
                          ("pass", 1, 1, 0),
                          ("qk", 1, 0, 1), ("v", 1, 1),
                          ("pass", 1, 0, 1),
                          ("qk", 1, 1, 1), ("pass", 1, 1, 1)]

                # htf loads ride the sync queue inside the last pass, each at
                # a ki where its group's AllToAll is long since done (so the
                # trigger never stalls the queue); ALL projections run in the
                # tail — mid-stream proj matmuls head-of-line block the PE on
                # peer skew, which perversely makes the fast cores finish last
                proj_sched = {
                    8: [(4, ("htf", 0)), (8, ("htf", 1)), (12, ("htf", 2))],
                }

                prev = None      # (b, lh, pas, stash, hacc)
                n_pass = sum(1 for ev in events if ev[0] == "pass")
                seen = 0
                for ev in events:
                    if ev[0] == "qk":
                        emit_qk_pair(ev[1], ev[2], ev[3])
                        continue
                    if ev[0] == "v":
                        emit_v_half(ev[1], ev[2])
                        continue
                    b_, lh, pas = ev[1], ev[2], ev[3]
                    seen += 1
                    last = (seen == n_pass)
                    qcs = (2 * pas, 2 * pas + 1)
                    kmax = (qcs[1] + 1) * (QB // KB)
                    hacc = {qc: hps.tile([65, QB], fp32, tag="hacc",
                                         name="hacc") for qc in qcs}
                    # interleave prev pass's PV matmuls between our ST blocks;
                    # in the final pass, drain prev early, then overlap our
                    # own PV behind a lag instead of leaving it for the flush
                    pv_todo = list(prev[3]) if prev is not None else []
                    horizon = 4 if last else kmax
                    stash = []
                    pt8 = None
                    acts = dict()
                    for ki, act in proj_sched.get(seen, []):
                        acts.setdefault(ki, []).append(act)
                    for ki in range(kmax):
                        for act in acts.get(ki, []):
                            emit_htf(act[1], nc.sync)
                        if pas == 1 and ki < 8:
                            # full-block range: paired fp8 exps feeding
                            # DoubleRow PV
                            if ki % 2 == 0:
                                pt8 = ptp.tile([128, 2048], fp8, tag="pt",
                                               name="pt8")
                            rows = slice(lh * 64, (lh + 1) * 64)
                            st = stps.tile([128, 1024], fp32, tag="st",
                                           name="st")
                            for qc in (2, 3):
                                nc.tensor.matmul(
                                    st[:, (qc - 2) * QB:(qc - 1) * QB],
                                    qku[b_][rows,
                                            L + ki * KB:L + (ki + 1) * KB],
                                    qku[b_][rows, qc * QB:(qc + 1) * QB],
                                    start=True, stop=True)
                            nc.scalar.activation(
                                pt8[:, (ki % 2) * 1024:(ki % 2 + 1) * 1024],
                                st[:], mybir.ActivationFunctionType.Exp,
                                bias=ebias[:])
                            if ki % 2 == 1:
                                stash.append((ki - 1, "DR", pt8))
                        else:
                            stash.append(st_block(b_, lh, pas, ki))
                        if ki < horizon and pv_todo:
                            take = -(-len(pv_todo) // (horizon - ki))
                            for _ in range(take):
                                pv_mms(prev[0], prev[1], prev[2], prev[4],
                                       pv_todo.pop(0))
                        if last and ki == horizon and prev is not None:
                            for qc in (2 * prev[2], 2 * prev[2] + 1):
                                norm_one(prev[0], prev[1], qc, prev[4][qc])
                        if last and ki > horizon and stash:
                            e = stash.pop(0)
                            pv_mms(b_, lh, pas, hacc, e)
                            if e[1] != "DR":
                                for qc in qcs:
                                    if e[0] == (qc + 1) * (QB // KB) - 1:
                                        norm_one(b_, lh, qc, hacc[qc])
                    if not last and prev is not None:
                        for qc in (2 * prev[2], 2 * prev[2] + 1):
                            norm_one(prev[0], prev[1], qc, prev[4][qc])
                        if prev[1] == 1:
                            # both lh halves of this group now shipped
                            fire_group(prev[0] * 2 + prev[2])
                    prev = (b_, lh, pas, stash, hacc)
                # final flush: PV the remaining stash, norm each q-chunk as
                # soon as its accumulation stops, then fire the last group
                normed = set()
                for e in prev[3]:
                    pv_mms(prev[0], prev[1], prev[2], prev[4], e)
                    if e[1] != "DR":
                        for qc in (2 * prev[2], 2 * prev[2] + 1):
                            if e[0] == (qc + 1) * (QB // KB) - 1:
                                norm_one(prev[0], prev[1], qc, prev[4][qc])
                                normed.add(qc)
                fire_group(3)

                # ---- tail: all four output projections ----
                # G0-G2's h tiles are already in SBUF; their projections fill
                # the PE while G3's AllToAll completes, then G3 projects.
                # htf3 is the only sync op left, out-DMAs alternate
                # scalar/gpsimd so nothing queues behind the CC3 wait
                emit_htf(3, nc.sync)
                for G in range(4):
                    emit_proj_half(G, 0, nc.scalar)
                    emit_proj_half(G, 1, nc.gpsimd)

    nc.compile()
    return nc


def _host_inputs(x, Wqkv, Wout):
    """Build the 8 per-core input maps (all fp16)."""
    x = np.asarray(x, dtype=np.float32)
    Wqkv = np.asarray(Wqkv, dtype=np.float32)
    Wout = np.asarray(Wout, dtype=np.float32)

    xT = np.ascontiguousarray(x.reshape(M, D).T).astype(np.float16)
    woutT = np.ascontiguousarray(Wout.T).astype(np.float16)

    scale = HD ** -0.5
    inv = ROPE_BASE ** (-np.arange(32, dtype=np.float64) / 32.0)
    l = np.arange(L, dtype=np.float64)
    ang = l[None, :] * inv[:, None]                      # [32, L]
    cosT = np.tile(np.cos(ang), (4, 1)).astype(np.float16)   # [128, L]
    sgn = np.repeat([1.0, -1.0, 1.0, -1.0], 32)[:, None]
    sinT = (np.tile(np.sin(ang), (4, 1)) * sgn).astype(np.float16)

    in_maps = []
    for c in range(N_CORES):
        a = HPC * c
        cols = []
        cols.append(Wqkv[HD * a:HD * (a + 1), :] * scale)          # q_a
        cols.append(Wqkv[D + HD * a:D + HD * (a + 1), :])          # k_a
        cols.append(Wqkv[D + HD * (a + 1):D + HD * (a + 2), :])    # k_{a+1}
        cols.append(Wqkv[HD * (a + 1):HD * (a + 2), :] * scale)    # q_{a+1}
        cols.append(Wqkv[2 * D + HD * a:2 * D + HD * (a + 1), :])  # v_a
        cols.append(Wqkv[2 * D + HD * (a + 1):2 * D + HD * (a + 2), :])
        wqkvT = np.ascontiguousarray(
            np.concatenate(cols, 0).T).astype(np.float16)
        in_maps.append({"xT": xT, "wqkvT": wqkvT, "woutT": woutT,
                        "cosT": cosT, "sinT": sinT})
    return in_maps


def kernel(x, Wqkv, Wout, _trace=False):
    global _NC
    if _NC is None:
        _NC = _build_nc()
    in_maps = _host_inputs(x, Wqkv, Wout)
    res = run_bass_kernel_spmd(_NC, in_maps, core_ids=list(range(N_CORES)),
                               trace=_trace)
    # core d's out rows [G*128:(G+1)*128] hold global tokens G*1024 + d*128
    flat = np.empty((M, D), dtype=np.float32)
    for d in range(N_CORES):
        o = res.results[d]["out"]
        for G in range(4):
            flat[G * 1024 + d * 128:G * 1024 + (d + 1) * 128] = \
                o[G * 128:(G + 1) * 128].astype(np.float32)
    full = flat.reshape(B, L, D)
    if _trace:
        kernel.last_results = res
    return full



# revision 28
# speedup vs baseline: 1.3099x; 1.3099x over previous
"""Causal MHA (B=2, L=2048, D=1024, 16 heads, RoPE) on 8 Trainium2 NeuronCores.

Strategy: tensor-parallel over heads (2 heads/core).
 - Host: transpose x -> x^T, build per-core W_qkv^T slices (q cols pre-scaled by
   1/sqrt(hd)), W_out^T, and RoPE cos/sin tables ([128, L], shared by both
   batches); everything fp16 for matmul operands (fp32 PSUM accumulation).
 - Device per core, fully software-pipelined at emission level:
     * inputs stream in consumption order (wq, cos/sin, x^T quarter-tiles,
       wout last) so the first attention pass can start ~15us in;
     * QK projection bursts are interleaved with attention passes; RoPE
       multiplies read the projection PSUM tile directly (no ACT copy);
     * attention pass: S^T = K^T.T @ Q^T -> exp (ACT) -> PV; the previous
       pass's PV matmuls are interleaved between the current pass's ST
       matmuls so the PE fills the stalls while exp paces the pipeline;
     * ones-column in V' yields softmax denominators; no max subtraction.
 - AllToAll pipelined in 4 per-1024-token groups: each group's collective
   fires mid-attention as soon as both head-halves of its chunk pair are
   normalized, so the redistribution and most of the output projection
   overlap later attention passes; only the last group's (small) collective
   and 128-token projection remain in the tail.
 - Host: scatter the per-core [4 x 128 tokens, 1024] chunks into the full
   output.
"""

import numpy as np

import concourse.bass as bass
import concourse.mybir as mybir
import concourse.tile as tile
from concourse import bacc
from concourse.bass_utils import run_bass_kernel_spmd

B, L, D, NH, HD = 2, 2048, 1024, 16, 64
ROPE_BASE = 10000.0
N_CORES = 8
HPC = NH // N_CORES          # heads per core = 2
M = B * L                    # 4096 tokens
MCH = 512                    # m-chunk (proj free dim)
KT = D // 128                # 8 contraction tiles
QB = 512                     # q chunk in attention
KB = 128                     # k block in attention
NKB = L // KB                # 16
NQC = L // QB                # 4
XCW = 1024                   # x^T column tile width
NXC = M // XCW               # 4 column tiles per k-tile
OH = MCH // 2                # out-proj half width (256)

fp16 = mybir.dt.float16
fp32 = mybir.dt.float32
fp8 = mybir.dt.float8e4

_NC = None


def _build_nc():
    nc = bacc.Bacc("TRN2", target_bir_lowering=False, debug=False,
                   num_devices=N_CORES)

    xT = nc.dram_tensor("xT", [D, M], fp16, kind="ExternalInput").ap()
    wqkvT = nc.dram_tensor("wqkvT", [D, 384], fp16, kind="ExternalInput").ap()
    woutT = nc.dram_tensor("woutT", [D, D], fp16, kind="ExternalInput").ap()
    cosT = nc.dram_tensor("cosT", [128, L], fp16, kind="ExternalInput").ap()
    sinT = nc.dram_tensor("sinT", [128, L], fp16, kind="ExternalInput").ap()
    out = nc.dram_tensor("out", [MCH, D], fp16, kind="ExternalOutput").ap()

    # 4 pipelined AllToAll groups, one per 1024-token chunk pair: group G
    # covers global tokens [G*1024, (G+1)*1024); destination core d receives
    # [all 1024 h-dims, tokens G*1024 + d*128 .. +128] and projects them.
    cc_in = [nc.dram_tensor(f"cc_in{g}", [D, 128], fp16) for g in range(4)]
    cc_out = [nc.dram_tensor(f"cc_out{g}", [D, 128], fp16) for g in range(4)]

    with tile.TileContext(nc) as tc:
        with tc.tile_pool(name="persist", bufs=1) as per, \
             tc.tile_pool(name="weights", bufs=1) as wp:
            # ---- constants first: they run on gpsimd/scalar immediately,
            # before those queues jam behind the input DMA flood ----
            # triangular 0/1 mask: keep where q-col >= k-row
            mask0 = per.tile([128, QB], fp16, tag="mask0")
            nc.gpsimd.memset(mask0[:], 1.0)
            nc.gpsimd.affine_select(
                out=mask0[:], in_=mask0[:], compare_op=mybir.AluOpType.is_ge,
                fill=0.0, base=0, channel_multiplier=-1, pattern=[[1, QB]],
            )
            ebias = per.tile([128, 1], fp32, tag="ebias")
            nc.gpsimd.memset(ebias[:], -3.0)
            # preload the exp table set during the input-DMA wait so the
            # first real exp skips the ~2.7us ACT_TABLE_LOAD
            dexp = per.tile([1, 1], fp16, tag="dexp")
            nc.scalar.activation(dexp[:], mask0[0:1, 0:1],
                                 mybir.ActivationFunctionType.Exp)
            # ---- input DMAs, in consumption order ----
            wq = [wp.tile([128, 384], fp16, tag=f"wq{k}", name=f"wq{k}")
                  for k in range(KT)]
            for k in range(KT):
                nc.sync.dma_start(wq[k][:], wqkvT[k * 128:(k + 1) * 128, :])
            # cos/sin split: the first half covers RoPE for token chunks 0-1,
            # so the second half can load after the first x column group
            cos_t = wp.tile([128, L], fp16, tag="cos")
            sin_t = wp.tile([128, L], fp16, tag="sin")
            nc.sync.dma_start(cos_t[:, 0:XCW], cosT[:, 0:XCW])
            nc.sync.dma_start(sin_t[:, 0:XCW], sinT[:, 0:XCW])
            # x^T as 8 k-tiles x 4 column tiles, streamed column-major so the
            # first 1024 tokens land first
            # alternate trigger queues: each DMA trigger costs ~650ns of
            # queue time, and 42 serial triggers (not HBM bandwidth) were
            # pacing the later column groups
            xt = [[None] * NXC for _ in range(KT)]
            for c in range(NXC):
                for k in range(KT):
                    t = wp.tile([128, XCW], fp16, tag=f"xt{k}_{c}",
                                name=f"xt{k}_{c}")
                    eng = nc.sync if k % 2 == 0 else nc.scalar
                    eng.dma_start(
                        t[:], xT[k * 128:(k + 1) * 128, c * XCW:(c + 1) * XCW])
                    xt[k][c] = t
                if c == 0:
                    nc.sync.dma_start(cos_t[:, XCW:L], cosT[:, XCW:L])
                    nc.sync.dma_start(sin_t[:, XCW:L], sinT[:, XCW:L])
            wo = [wp.tile([128, D], fp16, tag=f"wo{k}", name=f"wo{k}")
                  for k in range(KT)]
            for k in range(KT):
                nc.scalar.dma_start(wo[k][:], woutT[k * 128:(k + 1) * 128, :])

            def xcol(k, gc):
                """x^T slice for global 512-token chunk gc, k-tile k."""
                return xt[k][gc // 2][:, (gc % 2) * MCH:(gc % 2 + 1) * MCH]

            def xcol128(k, mt):
                """x^T slice for global 128-token tile mt, k-tile k."""
                return xt[k][mt // 8][:, (mt % 8) * 128:(mt % 8 + 1) * 128]

            # Q^T/K^T per batch: rows 0-63 head0, 64-127 head1;
            # cols 0:L = Q, L:2L = K
            qku = [per.tile([128, 2 * L], fp16, tag=f"qku{b}", name=f"qku{b}")
                   for b in range(B)]
            # V' per batch: per k-tile 130 cols = [v_h0(64) | 1 | v_h1(64) | 1]
            vt = [per.tile([128, (L // 128) * 130], fp16, tag=f"vt{b}",
                           name=f"vt{b}") for b in range(B)]
            # fp8 copy for DoubleRow PV over full-block pairs; per k-tile 144
            # cols = 2 head slots of 72 = [v(64) | 1 | pad(7)] so the pack-dim
            # byte-step (144) stays 16B-aligned
            vt8 = [per.tile([128, (L // 128) * 144], fp8, tag=f"vt8{b}",
                            name=f"vt8{b}") for b in range(B)]
            for b in range(B):
                nc.gpsimd.memset(vt[b][:], 1.0)
                nc.gpsimd.memset(vt8[b][:], 1.0)

            with tc.tile_pool(name="pp_ps", bufs=2, space="PSUM") as pps, \
                 tc.tile_pool(name="st_ps", bufs=2, space="PSUM") as stps, \
                 tc.tile_pool(name="h_ps", bufs=2, space="PSUM") as hps, \
                 tc.tile_pool(name="pt_sb", bufs=25) as ptp, \
                 tc.tile_pool(name="rope_sb", bufs=3) as rsb, \
                 tc.tile_pool(name="norm_sb", bufs=2) as nsb, \
                 tc.tile_pool(name="op_sb", bufs=3) as osb:

                # PE warm-up on the memset mask tile (no DMA dependency);
                # covers dispatch latency until the first x tiles land, so
                # the projection burst runs at full clock
                for _ in range(30):
                    wt = pps.tile([128, MCH], fp32, tag="pp", name="warm")
                    nc.tensor.matmul(wt[:], mask0[:, 0:128], mask0[:],
                                     start=True, stop=True)
                # then warm-ups paced by the x-tile DMAs themselves: activity
                # tracks the arrivals, so HAM never sees a >3.4us idle window
                # before the projection starts
                for k in range(KT):
                    for _ in range(2):
                        wt = pps.tile([128, MCH], fp32, tag="pp", name="warmx")
                        nc.tensor.matmul(wt[:], xt[k][0][:, 0:128],
                                         xt[k][0][:, 0:MCH],
                                         start=True, stop=True)

                def emit_qk_chunk(b_p, lh, mcb, early=False):
                    """QKV q/k projection + RoPE for one 512-token chunk.
                    Rope SBUF-SBUF moves always ride the gpsimd queue: it is
                    idle during startup (the sync queue is jammed behind the
                    8MB x-input flood) and never queues behind exps."""
                    dma_eng = nc.gpsimd
                    gc = b_p * NQC + mcb
                    qkp = pps.tile([128, MCH], fp32, tag="pp", name="qkp")
                    for k in range(KT):
                        nc.tensor.matmul(
                            qkp[:], wq[k][:, lh * 128:(lh + 1) * 128],
                            xcol(k, gc), start=(k == 0), stop=(k == KT - 1))
                    cs = slice(mcb * MCH, (mcb + 1) * MCH)
                    a_t = rsb.tile([128, MCH], fp16, tag="a")
                    c_t = rsb.tile([128, MCH], fp16, tag="c")
                    nc.vector.tensor_mul(a_t[:], qkp[:], cos_t[:, cs])
                    nc.vector.tensor_mul(c_t[:], qkp[:], sin_t[:, cs])
                    tmp = rsb.tile([128, MCH], fp16, tag="tmp")
                    for g in range(4):  # swap 32-row halves
                        src = (g ^ 1) * 32
                        dma_eng.dma_start(
                            tmp[g * 32:(g + 1) * 32, :],
                            c_t[src:src + 32, :])
                    bcol = mcb * MCH
                    # head0: q rows 0-63, k rows 64-127
                    # head1: k rows 0-63, q rows 64-127
                    qrows = slice(0, 64) if lh == 0 else slice(64, 128)
                    krows = slice(64, 128) if lh == 0 else slice(0, 64)
                    drows = slice(lh * 64, (lh + 1) * 64)
                    nc.vector.tensor_add(
                        qku[b_p][drows, bcol:bcol + MCH],
                        a_t[qrows, :], tmp[qrows, :])
                    nc.vector.tensor_add(
                        tmp[krows, :], a_t[krows, :], tmp[krows, :])
                    dma_eng.dma_start(
                        qku[b_p][drows, L + bcol:L + bcol + MCH],
                        tmp[krows, :])

                def emit_v_tile(mt):
                    """V projection for one 128-token tile."""
                    vp = pps.tile([128, 128], fp32, tag="pp", name="vp")
                    for k in range(KT):
                        nc.tensor.matmul(
                            vp[:], xcol128(k, mt), wq[k][:, 256:384],
                            start=(k == 0), stop=(k == KT - 1))
                    b_, kt_ = mt // (L // 128), mt % (L // 128)
                    dst = vt[b_][:, kt_ * 130:kt_ * 130 + 130]
                    dst = dst.rearrange("p (g c) -> p g c", g=2)[:, :, 0:64]
                    nc.vector.tensor_copy(
                        dst, vp[:].rearrange("p (g c) -> p g c", g=2))
                    dst8 = vt8[b_][:, kt_ * 144:kt_ * 144 + 144]
                    dst8 = dst8.rearrange("p (g c) -> p g c", g=2)[:, :, 0:64]
                    nc.vector.tensor_copy(
                        dst8, vp[:].rearrange("p (g c) -> p g c", g=2))

                def emit_qk_pair(b_p, lh, pair):
                    early = (b_p == 0 and pair == 0)
                    for mcb in (2 * pair, 2 * pair + 1):
                        emit_qk_chunk(b_p, lh, mcb, early=early)

                def emit_v_half(b_p, pair):
                    for mt in range(b_p * 16 + pair * 8,
                                    b_p * 16 + pair * 8 + 8):
                        emit_v_tile(mt)

                def st_block(b_, lh, pas, ki):
                    """ST matmuls + exp + mask for one k-block; returns stash
                    entry."""
                    rows = slice(lh * 64, (lh + 1) * 64)
                    qcs = (2 * pas, 2 * pas + 1)
                    qlo = max(qcs[0], ki // (QB // KB))
                    pofs = []
                    for qc in range(qlo, qcs[1] + 1):
                        diag = (qc == ki // (QB // KB))
                        off = (ki % (QB // KB)) * KB if diag else 0
                        w = QB - off
                        # fixed 512-stride slots: each matmul stays in one
                        # PSUM bank; diag slack unread downstream
                        pofs.append((qc, (qc - qlo) * QB, w,
                                     qc * QB + off, off))
                    p0 = (qcs[1] + 1 - qlo) * QB
                    st = stps.tile([128, p0], fp32, tag="st", name="st")
                    pt = ptp.tile([128, p0], fp16, tag="pt", name="pt")
                    for qc, ps, w, qs, off in pofs:
                        nc.tensor.matmul(
                            st[:, ps:ps + w],
                            qku[b_][rows, L + ki * KB:L + (ki + 1) * KB],
                            qku[b_][rows, qs:qs + w],
                            start=True, stop=True)
                    # exp(x-3): uniform rescale (cancels in the softmax
                    # ratio) keeping values inside fp8e4 range for the
                    # DoubleRow path
                    nc.scalar.activation(
                        pt[:], st[:], mybir.ActivationFunctionType.Exp,
                        bias=ebias[:])
                    if ki // (QB // KB) == qlo:
                        # mask differs from 1 only in the first KB cols
                        w0 = min(pofs[0][2], KB)
                        nc.vector.tensor_mul(
                            pt[:, 0:w0], pt[:, 0:w0], mask0[:, 0:w0])
                    return (ki, pofs, pt)

                def norm_one(b_, lh, qc, ha):
                    """Normalize one q-chunk (denominator in ha row 64) and
                    ship h^T to its AllToAll group buffer. Emitted at the
                    accumulation's stop matmul, so the ACT row-move queues
                    only a few exps deep."""
                    dsb = nsb.tile([1, QB], fp32, tag="dsb")
                    nc.scalar.copy(dsb[:], ha[64:65, :])
                    recip = nsb.tile([1, QB], fp32, tag="recip")
                    nc.vector.reciprocal_approx_fast(recip[:], dsb[:])
                    rb = nsb.tile([64, QB], fp32, tag="rb")
                    nc.gpsimd.partition_broadcast(rb[:], recip[:])
                    ht = nsb.tile([64, QB], fp16, tag="ht")
                    nc.vector.tensor_mul(ht[:], ha[0:64, :], rb[:])
                    # group G, chunk half e: dest core d = 4e+c takes token
                    # cols [c*128, (c+1)*128); our 64 rows land at global
                    # h-dim rows d*128 + lh*64 of cc_in[G]
                    G = b_ * 2 + qc // 2
                    e = qc % 2
                    dst = cc_in[G].ap().rearrange(
                        "(d p) t -> p d t", p=128)[
                            lh * 64:lh * 64 + 64, 4 * e:4 * e + 4, :]
                    nc.sync.dma_start(
                        dst, ht[:, :].rearrange("p (c t) -> p c t", t=128))

                def pv_mms(b_, lh, pas, hacc, entry):
                    """PV matmuls for one stash entry of pass (b_, lh, pas)."""
                    ki, pofs, pt = entry
                    if pofs == "DR":
                        # fp8 DoubleRow: one matmul contracts k-blocks ki and
                        # ki+1 (256 rows) at 0.5 cycles/col
                        vsl8 = vt8[b_].rearrange("p (k c) -> p k c", c=144)[
                            :, ki:ki + 2, lh * 72:lh * 72 + 65]
                        p2 = pt.rearrange("p (j c) -> p j c", j=2)
                        for qc in (2, 3):
                            nc.tensor.matmul(
                                hacc[qc][:, 0:QB], vsl8,
                                p2[:, :, (qc - 2) * QB:(qc - 1) * QB],
                                start=(ki == 0), stop=False,
                                perf_mode=mybir.MatmulPerfMode.DoubleRow)
                        return
                    vsl = vt[b_][:, ki * 130 + lh * 65:ki * 130 + lh * 65 + 65]
                    for qc, ps, w, qs, off in pofs:
                        nc.tensor.matmul(
                            hacc[qc][:, off:off + w], vsl, pt[:, ps:ps + w],
                            start=(ki == 0),
                            stop=(ki == (qc + 1) * (QB // KB) - 1))

                def fire_group(G):
                    """Trigger the AllToAll for token group G (inputs are the
                    norm DMAs already emitted); runs async on the CC engine
                    while attention continues."""
                    nc.gpsimd.collective_compute(
                        "AllToAll", mybir.AluOpType.bypass,
                        replica_groups=[list(range(N_CORES))],
                        ins=[cc_in[G].ap().opt()],
                        outs=[cc_out[G].ap().opt()],
                    )

                htf_of = {}

                def emit_htf(G, eng=None):
                    """One-shot SBUF load of the received h^T for group G
                    (single 256KB DMA; waits on the group's AllToAll
                    semaphore inside the chosen DMA queue)."""
                    htf = osb.tile([128, 1024], fp16, tag="htf", name="htf")
                    src = cc_out[G].ap().rearrange("(g p) t -> p g t", p=128)
                    (eng or nc.sync).dma_start(
                        htf[:].rearrange("p (g t) -> p g t", t=128), src)
                    htf_of[G] = htf

                def emit_proj_half(G, dh, out_eng=None):
                    """Out-projection of group G's 128 tokens, dims half dh.
                    h tile stationary, Wout moving -> stream-bound N=512."""
                    htf = htf_of[G]
                    op = pps.tile([128, 512], fp32, tag="pp", name="proj")
                    for k in range(KT):
                        nc.tensor.matmul(
                            op[:], htf[:, k * 128:(k + 1) * 128],
                            wo[k][:, dh * 512:(dh + 1) * 512],
                            start=(k == 0), stop=(k == KT - 1))
                    ot = nsb.tile([128, 512], fp16, tag="ot")
                    nc.vector.tensor_copy(ot[:], op[:])
                    (out_eng or nc.sync).dma_start(
                        out[G * 128:(G + 1) * 128, dh * 512:(dh + 1) * 512],
                        ot[:])

                # ---- interleaved projection pieces + attention passes ----
                # lh-adjacent pass order completes each 1024-token group as
                # early as possible so its AllToAll overlaps later passes; qk
                # pieces land just before the pass that needs them so the
                # first ST only queues behind 16 projection matmuls
                # b1's first qk pair is pulled one slot earlier so its RoPE
                # vector/gpsimd work overlaps the (vector-light) DR pass p4
                # instead of stalling p5's first STs
                events = [("qk", 0, 0, 0), ("pass", 0, 0, 0),
                          ("qk", 0, 1, 0), ("v", 0, 0),
                          ("pass", 0, 1, 0),
                          ("qk", 0, 0, 1), ("v", 0, 1),
                          ("pass", 0, 0, 1),
                          ("qk", 0, 1, 1), ("qk", 1, 0, 0),
                          ("pass", 0, 1, 1),
                          ("pass", 1, 0, 0),
                          ("qk", 1, 1, 0), ("v", 1, 0),
                          ("pass", 1, 1, 0),
                          ("qk", 1, 0, 1), ("v", 1, 1),
                          ("pass", 1, 0, 1),
                          ("qk", 1, 1, 1), ("pass", 1, 1, 1)]

                # htf loads ride the sync queue inside the last pass, each at
                # a ki where its group's AllToAll is long since done (so the
                # trigger never stalls the queue); ALL projections run in the
                # tail — mid-stream proj matmuls head-of-line block the PE on
                # peer skew, which perversely makes the fast cores finish last
                proj_sched = {
                    8: [(4, ("htf", 0)), (8, ("htf", 1)), (12, ("htf", 2))],
                }

                prev = None      # (b, lh, pas, stash, hacc)
                n_pass = sum(1 for ev in events if ev[0] == "pass")
                seen = 0
                for ev in events:
                    if ev[0] == "qk":
                        emit_qk_pair(ev[1], ev[2], ev[3])
                        continue
                    if ev[0] == "v":
                        emit_v_half(ev[1], ev[2])
                        continue
                    b_, lh, pas = ev[1], ev[2], ev[3]
                    seen += 1
                    last = (seen == n_pass)
                    qcs = (2 * pas, 2 * pas + 1)
                    kmax = (qcs[1] + 1) * (QB // KB)
                    hacc = {qc: hps.tile([65, QB], fp32, tag="hacc",
                                         name="hacc") for qc in qcs}
                    # interleave prev pass's PV matmuls between our ST blocks;
                    # in the final pass, drain prev early, then overlap our
                    # own PV behind a lag instead of leaving it for the flush
                    pv_todo = list(prev[3]) if prev is not None else []
                    horizon = 4 if last else kmax
                    stash = []
                    pt8 = None
                    acts = dict()
                    for ki, act in proj_sched.get(seen, []):
                        acts.setdefault(ki, []).append(act)
                    for ki in range(kmax):
                        for act in acts.get(ki, []):
                            emit_htf(act[1], nc.sync)
                        if pas == 1 and ki < 8:
                            # full-block range: paired fp8 exps feeding
                            # DoubleRow PV
                            if ki % 2 == 0:
                                pt8 = ptp.tile([128, 2048], fp8, tag="pt",
                                               name="pt8")
                            rows = slice(lh * 64, (lh + 1) * 64)
                            st = stps.tile([128, 1024], fp32, tag="st",
                                           name="st")
                            for qc in (2, 3):
                                nc.tensor.matmul(
                                    st[:, (qc - 2) * QB:(qc - 1) * QB],
                                    qku[b_][rows,
                                            L + ki * KB:L + (ki + 1) * KB],
                                    qku[b_][rows, qc * QB:(qc + 1) * QB],
                                    start=True, stop=True)
                            nc.scalar.activation(
                                pt8[:, (ki % 2) * 1024:(ki % 2 + 1) * 1024],
                                st[:], mybir.ActivationFunctionType.Exp,
                                bias=ebias[:])
                            if ki % 2 == 1:
                                stash.append((ki - 1, "DR", pt8))
                        else:
                            stash.append(st_block(b_, lh, pas, ki))
                        if ki < horizon and pv_todo:
                            take = -(-len(pv_todo) // (horizon - ki))
                            for _ in range(take):
                                pv_mms(prev[0], prev[1], prev[2], prev[4],
                                       pv_todo.pop(0))
                        if last and ki == horizon and prev is not None:
                            for qc in (2 * prev[2], 2 * prev[2] + 1):
                                norm_one(prev[0], prev[1], qc, prev[4][qc])
                        if last and ki > horizon and stash:
                            e = stash.pop(0)
                            pv_mms(b_, lh, pas, hacc, e)
                            if e[1] != "DR":
                                for qc in qcs:
                                    if e[0] == (qc + 1) * (QB // KB) - 1:
                                        norm_one(b_, lh, qc, hacc[qc])
                    if not last and prev is not None:
                        for qc in (2 * prev[2], 2 * prev[2] + 1):
                            norm_one(prev[0], prev[1], qc, prev[4][qc])
                        if prev[1] == 1:
                            # both lh halves of this group now shipped
                            fire_group(prev[0] * 2 + prev[2])
                    prev = (b_, lh, pas, stash, hacc)
                # final flush: PV the remaining stash, norm each q-chunk as
                # soon as its accumulation stops, then fire the last group
                normed = set()
                for e in prev[3]:
                    pv_mms(prev[0], prev[1], prev[2], prev[4], e)
                    if e[1] != "DR":
                        for qc in (2 * prev[2], 2 * prev[2] + 1):
                            if e[0] == (qc + 1) * (QB // KB) - 1:
                                norm_one(prev[0], prev[1], qc, prev[4][qc])
                                normed.add(qc)
                fire_group(3)

                # ---- tail: all four output projections ----
                # G0-G2's h tiles are already in SBUF; their projections fill
                # the PE while G3's AllToAll completes, then G3 projects.
                # htf3 is the only sync op left, out-DMAs alternate
                # scalar/gpsimd so nothing queues behind the CC3 wait
                emit_htf(3, nc.sync)
                for G in range(4):
                    emit_proj_half(G, 0, nc.scalar)
                    emit_proj_half(G, 1, nc.scalar)

    nc.compile()
    return nc


def _host_inputs(x, Wqkv, Wout):
    """Build the 8 per-core input maps (all fp16)."""
    x = np.asarray(x, dtype=np.float32)
    Wqkv = np.asarray(Wqkv, dtype=np.float32)
    Wout = np.asarray(Wout, dtype=np.float32)

    xT = np.ascontiguousarray(x.reshape(M, D).T).astype(np.float16)
    woutT = np.ascontiguousarray(Wout.T).astype(np.float16)

    scale = HD ** -0.5
    inv = ROPE_BASE ** (-np.arange(32, dtype=np.float64) / 32.0)
    l = np.arange(L, dtype=np.float64)
    ang = l[None, :] * inv[:, None]                      # [32, L]
    cosT = np.tile(np.cos(ang), (4, 1)).astype(np.float16)   # [128, L]
    sgn = np.repeat([1.0, -1.0, 1.0, -1.0], 32)[:, None]
    sinT = (np.tile(np.sin(ang), (4, 1)) * sgn).astype(np.float16)

    in_maps = []
    for c in range(N_CORES):
        a = HPC * c
        cols = []
        cols.append(Wqkv[HD * a:HD * (a + 1), :] * scale)          # q_a
        cols.append(Wqkv[D + HD * a:D + HD * (a + 1), :])          # k_a
        cols.append(Wqkv[D + HD * (a + 1):D + HD * (a + 2), :])    # k_{a+1}
        cols.append(Wqkv[HD * (a + 1):HD * (a + 2), :] * scale)    # q_{a+1}
        cols.append(Wqkv[2 * D + HD * a:2 * D + HD * (a + 1), :])  # v_a
        cols.append(Wqkv[2 * D + HD * (a + 1):2 * D + HD * (a + 2), :])
        wqkvT = np.ascontiguousarray(
            np.concatenate(cols, 0).T).astype(np.float16)
        in_maps.append({"xT": xT, "wqkvT": wqkvT, "woutT": woutT,
                        "cosT": cosT, "sinT": sinT})
    return in_maps


def kernel(x, Wqkv, Wout, _trace=False):
    global _NC
    if _NC is None:
        _NC = _build_nc()
    in_maps = _host_inputs(x, Wqkv, Wout)
    res = run_bass_kernel_spmd(_NC, in_maps, core_ids=list(range(N_CORES)),
                               trace=_trace)
    # core d's out rows [G*128:(G+1)*128] hold global tokens G*1024 + d*128
    flat = np.empty((M, D), dtype=np.float32)
    for d in range(N_CORES):
        o = res.results[d]["out"]
        for G in range(4):
            flat[G * 1024 + d * 128:G * 1024 + (d + 1) * 128] = \
                o[G * 128:(G + 1) * 128].astype(np.float32)
    full = flat.reshape(B, L, D)
    if _trace:
        kernel.last_results = res
    return full

